# revision 1
# baseline (speedup 1.0000x reference)
"""NT-Xent loss kernel for Trainium2, 8 NeuronCores.

Strategy (row-sharded similarity matrix):
  - Each core receives the full feature matrix, cyclically rolled by
    c*1024 rows, so every core's program is identical: its 1024 rows are
    always rolled-rows [0, 1024), its positive columns always [4096, 5120).
  - On device: normalize all 8192 rows (DVE square+reduce, ACT ln/exp for
    rsqrt), PE-transpose into feature-major zbT [128, 8192] (float32r),
    then 128 matmuls of [128,128]x[128,512] produce the core's 1024x8192
    similarity block in PSUM.  ScalarE computes exp(10*sim) with a fused
    per-row accumulation (accum_out) giving row sums for free; diagonal and
    positive entries are extracted with identity-mask fused multiply-reduce.
  - loss_row = ln(rowsum - exp_diag) - ln(exp_pos); per-core [128, 8] loss
    tile is DMA'd out; the host sums partials and divides by N.

  The column dimension is processed in 4 groups of 2048; the per-group
  preamble (load/normalize/transpose) is interleaved with the previous
  group's main loop so the ScalarE exp stream (the bottleneck at ~66us)
  starts ~12us in and never stalls.
"""

import os

import numpy as np

N = 8192
D = 128
NCORES = 8
RPC = N // NCORES          # rows per core = 1024
G = 4                      # zbT column groups
GCOLS = N // G             # 2048 columns per group
RT = RPC // 128            # row tiles per core = 8
ESC = 10.0                 # 1 / temperature

_CACHE = {}
LAST_RESULTS = None


def _patch_act_tables():
    """Force Exp/Ln onto the combined natural_log_exp_and_others table set.

    The greedy table-load pass otherwise alternates between exp-only and
    ln-only sets (one ~2.7us table load per switch, 11 loads for this
    kernel).  Stripping Exp/Ln from the competing sets leaves exactly one
    set that can serve them, so a single load covers the whole kernel.
    Set ids stay valid because the dict keeps all entries in json order.
    """
    if _CACHE.get("act_patched"):
        return
    import functools

    import concourse.bacc as bacc_mod
    import concourse.bass_interp as interp_mod
    import concourse.hw_specs as hw_specs
    import concourse.mybir as mybir

    AF = mybir.ActivationFunctionType
    orig = hw_specs.get_activation_tables

    @functools.cache
    def patched(arch):
        out = {}
        for name, funcs in orig(arch).items():
            if name != "natural_log_exp_and_others":
                funcs = funcs - {AF.Exp, AF.Ln}
            out[name] = funcs
        return out

    hw_specs.get_activation_tables = patched
    bacc_mod.get_activation_tables = patched
    interp_mod.get_activation_tables = patched
    _CACHE["act_patched"] = True


def _patch_ldw_opt():
    """Let walrus dedup consecutive identical LDWEIGHTS (off by default
    upstream); our inner loop reuses each stationary operand 4x."""
    if _CACHE.get("ldw_patched") or not os.environ.get("KERNEL_LDW_OPT"):
        return
    import concourse.bass_utils as bu

    orig = bu.run_command

    def run2(argv, **kw):
        argv = [
            "--enable-ldw-opt=true" if a == "--enable-ldw-opt=false" else a
            for a in argv
        ]
        return orig(argv, **kw)

    bu.run_command = run2
    _CACHE["ldw_patched"] = True


def _build():
    import concourse.mybir as mybir
    import concourse.tile as tile
    from concourse import bacc

    _patch_act_tables()
    _patch_ldw_opt()

    f32 = mybir.dt.float32
    bf16 = mybir.dt.bfloat16
    AX = mybir.AxisListType
    OP = mybir.AluOpType
    AF = mybir.ActivationFunctionType

    nc = bacc.Bacc(
        "TRN2",
        target_bir_lowering=False,
        debug=False,
        enable_asserts=False,
        num_devices=NCORES,
    )
    x = nc.dram_tensor("x", [N, D], f32, kind="ExternalInput").ap()
    ident_in = nc.dram_tensor("ident", [128, 128], f32, kind="ExternalInput").ap()
    out = nc.dram_tensor("loss_parts", [128, RT], f32, kind="ExternalOutput").ap()

    with tile.TileContext(nc) as tc:
        with (
            tc.tile_pool(name="const", bufs=1) as constp,
            tc.tile_pool(name="xin", bufs=4) as xinp,
            tc.tile_pool(name="zb", bufs=2) as zbp,
            tc.tile_pool(name="zbT", bufs=4) as zbTp,
            tc.tile_pool(name="expp", bufs=3) as expp,
            tc.tile_pool(name="small", bufs=2) as smallp,
            tc.tile_pool(name="acc", bufs=1) as accp,
            tc.tile_pool(name="psum", bufs=2, space="PSUM") as psump,
        ):
            ident = constp.tile([128, 128], f32, tag="ident")
            nc.sync.dma_start(out=ident[:], in_=ident_in)
            identb = constp.tile([128, 128], bf16, tag="identb")
            nc.vector.tensor_copy(identb[:], ident[:])

            # Touch Ln+Exp early so the ACT table load overlaps the input DMA.
            warm = constp.tile([128, 1], f32, tag="warm")
            nc.vector.memset(warm[:], 1.0)
            nc.scalar.activation(warm[:], warm[:], AF.Ln)
            nc.scalar.activation(warm[:], warm[:], AF.Exp)

            # Per-partition eps^2 bias tile for the norm clamp.
            eps2 = constp.tile([128, 1], f32, tag="eps2")
            nc.vector.memset(eps2[:], 1e-16)

            # Accumulators, live for the whole kernel.
            racc = accp.tile([128, RT * G], f32, tag="racc")  # row sums
            dall = accp.tile([128, RT], f32, tag="dall")      # exp(diag)
            pall = accp.tile([128, RT], f32, tag="pall")      # exp(pos)

            zbT = [None] * G
            rnos = [None] * G
            xgs = [None] * G
            prev_copy = [None]

            def norm_part(g):
                """Load group g and compute its 2048 reciprocal row norms.

                The tiny Ln/Exp land in ACT program order wherever this is
                called, so callers inject it mid-way through the previous
                group's exp stream (in-order engine; placing it earlier would
                head-of-line block, later would stall the next group).
                """
                from bass_rust import add_dep_helper

                xg = xinp.tile([128, GCOLS], f32, tag="xg")
                for q in range(4):
                    src = x[g * GCOLS + q * 512 : g * GCOLS + (q + 1) * 512, :]
                    src = src.rearrange("(s p) d -> p s d", p=128)
                    dst = xg[:, q * 512 : (q + 1) * 512].rearrange(
                        "p (s d) -> p s d", s=4
                    )
                    # For group 0 (the critical head), stream on both HWDGE
                    # rings (SP and ACT) in parallel; later groups stay on
                    # the SP ring to keep the ACT queue free for exps.
                    eng = nc.scalar if (g == 0 and q % 2 == 1) else nc.sync
                    eng.dma_start(out=dst, in_=src)

                # Square+reduce in halves so the first half starts while the
                # second half's DMA is still in flight.
                sq = zbp.tile([128, GCOLS], f32, tag="sq")
                nsq = smallp.tile([128, 16], f32, tag="nsq")
                for h in range(2):
                    hs = slice(h * 1024, (h + 1) * 1024)
                    sqi = nc.vector.tensor_mul(sq[:, hs], xg[:, hs], xg[:, hs])
                    if prev_copy[0] is not None:
                        # Keep the DVE strictly group-ordered: group g's norm
                        # work must not steal DVE slots from group g-1's
                        # critical chain.
                        add_dep_helper(
                            sqi.ins, prev_copy[0].ins, sync=False,
                            reason="serialize preamble DVE across groups",
                        )
                    nc.vector.tensor_reduce(
                        nsq[:, h * 8 : (h + 1) * 8],
                        sq[:, hs].rearrange("p (s d) -> p s d", s=8),
                        axis=AX.X,
                        op=OP.add,
                    )
                # rnorm = exp(-0.5 * ln(nsq + eps^2)); the bias stands in for
                # the reference's max(norm, eps) clamp.
                lnv = smallp.tile([128, 16], f32, tag="lnv")
                nc.scalar.activation(lnv[:], nsq[:], AF.Ln, bias=eps2[:, 0:1])
                rno = smallp.tile([128, 16], f32, tag="rno")
                nc.scalar.activation(rno[:], lnv[:], AF.Exp, scale=-0.5)
                rnos[g] = rno
                xgs[g] = xg

            def zb_part(g):
                """Scale group g by its row norms and transpose into zbT.

                Pipelined in 512-column chunks (scale -> transpose -> copy)
                so the psum->sbuf copies overlap the scales and zbT becomes
                ready as early as possible.
                """
                xg, rno = xgs[g], rnos[g]
                zbg = zbp.tile([128, GCOLS], bf16, tag="zbg")
                pt = psump.tile([128, GCOLS], bf16, tag="pt")
                zt = zbTp.tile([128, GCOLS], bf16, tag="zbT")
                for q in range(4):
                    for j in range(4):
                        s = 4 * q + j
                        nc.vector.tensor_scalar_mul(
                            zbg[:, s * 128 : (s + 1) * 128],
                            xg[:, s * 128 : (s + 1) * 128],
                            rno[:, s : s + 1],
                        )
                        nc.tensor.transpose(
                            pt[:, s * 128 : (s + 1) * 128],
                            zbg[:, s * 128 : (s + 1) * 128],
                            identb[:],
                        )
                    prev_copy[0] = nc.vector.tensor_copy(
                        zt[:, q * 512 : (q + 1) * 512],
                        pt[:, q * 512 : (q + 1) * 512],
                    )
                zbT[g] = zt

            def mm_exp(m, g):
                pt = psump.tile([128, GCOLS], f32, tag="pt")
                lhs = zbT[0][:, m * 128 : (m + 1) * 128]
                for k in range(4):
                    nc.tensor.matmul(
                        pt[:, k * 512 : (k + 1) * 512],
                        lhs,
                        zbT[g][:, k * 512 : (k + 1) * 512],
                    )
                et = expp.tile([128, GCOLS], f32, tag="et")
                nc.scalar.activation(
                    et[:], pt[:], AF.Exp, scale=ESC,
                    accum_out=racc[:, m * G + g : m * G + g + 1],
                )
                if g == 0 or g == 2:
                    # diag block: cols [m*128, +128) of group 0;
                    # positive block: same offset in group 2.
                    # (tensor_tensor_reduce would fuse these, but that custom
                    # op faults at runtime on this stack -- use mul+reduce.)
                    tgt = dall if g == 0 else pall
                    scr = smallp.tile([128, 128], f32, tag="scrB")
                    nc.vector.tensor_mul(
                        scr[:], et[:, m * 128 : (m + 1) * 128], ident[:]
                    )
                    nc.vector.tensor_reduce(
                        tgt[:, m : m + 1], scr[:], axis=AX.X, op=OP.add
                    )

            # Interleave: group g+1's norms are injected after the 4th exp of
            # group g (mid-stream on the in-order ScalarE), its scale+transpose
            # right after the stream, so zbT[g+1] is always ready on time.
            norm_part(0)
            zb_part(0)
            for g in range(G):
                for m in range(RT):
                    mm_exp(m, g)
                    if m == 3 and g + 1 < G:
                        norm_part(g + 1)
                    if m == 5 and g + 1 < G:
                        zb_part(g + 1)

            # ---- epilogue: loss = ln(rowsum - exp_diag) - ln(exp_pos) ----
            tot = smallp.tile([128, RT], f32, tag="tot")
            nc.vector.tensor_reduce(
                tot[:],
                racc[:].rearrange("p (m g) -> p m g", g=G),
                axis=AX.X,
                op=OP.add,
            )
            ndall = smallp.tile([128, RT], f32, tag="ndall")
            nc.vector.tensor_sub(ndall[:], tot[:], dall[:])
            lnd = smallp.tile([128, RT], f32, tag="lnd")
            nc.scalar.activation(lnd[:], ndall[:], AF.Ln)
            lnp = smallp.tile([128, RT], f32, tag="lnp")
            nc.scalar.activation(lnp[:], pall[:], AF.Ln)
            lt = smallp.tile([128, RT], f32, tag="lt")
            nc.vector.tensor_sub(lt[:], lnd[:], lnp[:])
            nc.sync.dma_start(out=out, in_=lt[:])

    nc.compile()
    return nc


def _get_nc():
    if "nc" not in _CACHE:
        _CACHE["nc"] = _build()
    return _CACHE["nc"]


def kernel(stacked_batch: np.ndarray) -> np.ndarray:
    global LAST_RESULTS
    from concourse.bass_utils import run_bass_kernel_spmd

    nc = _get_nc()
    xf = np.ascontiguousarray(np.asarray(stacked_batch, dtype=np.float32))
    assert xf.shape == (N, D)

    ident = np.eye(128, dtype=np.float32)
    in_maps = [
        {"x": np.ascontiguousarray(np.roll(xf, -c * RPC, axis=0)), "ident": ident}
        for c in range(NCORES)
    ]
    res = run_bass_kernel_spmd(
        nc,
        in_maps,
        core_ids=list(range(NCORES)),
        trace=bool(os.environ.get("BASS_TRACE")),
    )
    LAST_RESULTS = res
    total = 0.0
    for c in range(NCORES):
        total += float(np.asarray(res.results[c]["loss_parts"], dtype=np.float64).sum())
    return np.float32(total / N)

